# revision 1
# baseline (speedup 1.0000x reference)
"""Trainium2 Bass kernel for nn_Decorder_52467320488266 (retrieval_knn).

Per batch element (one per NeuronCore):
  a = f1 @ f2.T / TEMP                         [L, L] logits, fp32 PE matmul
  m = softmax(a, 0) * softmax(a, 1)
  mask = (m > 0.2) & mutual-argmax(m)
  out[i] = f1[i] - f2[j*_i] if matched else f1[i], transposed to [c, L]

Implemented without materializing m:
  m_ij = exp(2 a_ij - LSE_r_i - LSE_c_j).  Row/col top-8 (DVE Max8/MaxIndex)
  of the raw logits give exact-to-fp32 LSEs (rank-9+ terms < e^-80) and the
  candidate sets; candidates are re-ranked with the exact LSE corrections.
  Mutuality is checked by comparing the row-side max of u = 2a - LSE_r - LSE_c
  against the column-side max at j* (equal iff mutual; tolerance 2e-3 vs
  observed cross-matmul fp32 noise <5e-4 and true non-mutual gaps >5e-3).
  The final subtract happens in transposed space: f2 columns are gathered
  from f2.T via gpsimd indirect_copy, so no output transpose is needed.
"""

import sys
import numpy as np

for _p in ("/opt/trn_rl_repo",):
    if _p not in sys.path:
        sys.path.insert(0, _p)

B, L, C = 8, 4096, 128
NSTRIP = L // 128     # 32
NH = L // 2048        # 2 psum half-strips
TEMP = 0.1
LN_THRESH = float(np.log(0.2))
EPS_MUT = 2e-3

_NC_CACHE = {}


def _build_nc(reps=1):
    import concourse.bass as bass
    import concourse.bacc as bacc
    import concourse.tile as tile
    from concourse import mybir

    f32 = mybir.dt.float32
    u16 = mybir.dt.uint16
    AF = mybir.ActivationFunctionType
    OP = mybir.AluOpType
    X = mybir.AxisListType.X

    nc = bacc.Bacc()
    f1_d = nc.dram_tensor("f1", [L, C], f32, kind="ExternalInput")
    f2_d = nc.dram_tensor("f2", [L, C], f32, kind="ExternalInput")
    ident_d = nc.dram_tensor("ident", [128, 128], f32, kind="ExternalInput")
    mask16_d = nc.dram_tensor("mask16", [128, 16], f32, kind="ExternalInput")
    out_d = nc.dram_tensor("out", [128, L], f32, kind="ExternalOutput")
    # DRAM scratch for partition<->free layout bounces
    cvec_d = nc.dram_tensor("cvec", [L], f32)
    rvec_d = nc.dram_tensor("rvec", [L], f32)
    wvec_d = nc.dram_tensor("wvec", [L], f32)
    jvec_d = nc.dram_tensor("jvec", [L], u16)

    with tile.TileContext(nc) as tc:
        from contextlib import ExitStack

        with ExitStack() as ctx:
            pers = ctx.enter_context(tc.tile_pool(name="pers", bufs=1))
            big = ctx.enter_context(tc.tile_pool(name="big", bufs=6))
            tbl = ctx.enter_context(tc.tile_pool(name="tbl", bufs=1))
            sm = ctx.enter_context(tc.tile_pool(name="sm", bufs=2))
            psA = ctx.enter_context(
                tc.tile_pool(name="psA", bufs=2, space=bass.MemorySpace.PSUM)
            )

            for _rep in range(reps):
                # ---- persistent tiles
                f1t = pers.tile([128, L], f32, tag="f1t")   # f1.T  [c, i], unscaled
                f2t = pers.tile([128, L + 8], f32, tag="f2t")  # f2.T [c, j] + zero pad col
                ident = pers.tile([128, 128], f32, tag="ident")
                mask16 = pers.tile([128, 16], f32, tag="mask16")
                ones1 = pers.tile([1, 128], f32, tag="ones1")
                rv8a = pers.tile([128, 8 * NSTRIP], f32, tag="rv8a")    # row top8 vals
                ri8a = pers.tile([128, 8 * NSTRIP], u16, tag="ri8a")    # row top8 idx
                cv8a = pers.tile([128, 8 * NSTRIP], f32, tag="cv8a")    # col top8 vals
                ci8a = pers.tile([128, 8 * NSTRIP], u16, tag="ci8a")    # col top8 idx
                lser_c = pers.tile([128, NSTRIP], f32, tag="lser")      # LSE_row [p, s]
                lsec_c = pers.tile([128, NSTRIP], f32, tag="lsec")      # LSE_col
                ustar_c = pers.tile([128, NSTRIP], f32, tag="ustar")
                jstar_c = pers.tile([128, NSTRIP], f32, tag="jstar")
                jst_u16 = pers.tile([128, NSTRIP], u16, tag="jstu")
                cmw_c = pers.tile([128, NSTRIP], f32, tag="cmw")
                keep_c = pers.tile([128, NSTRIP], f32, tag="keep")
                idxsw = pers.tile([128, L // 16], u16, tag="idxsw")

                nc.sync.dma_start(ident[:], ident_d[:, :])
                nc.sync.dma_start(mask16[:], mask16_d[:, :])
                nc.gpsimd.memset(ones1[:], 1.0)

                # ---- transpose-load f1, f2 -> f1t, f2t (exact copies)
                nc.gpsimd.memset(f2t[:, L : L + 8], 0.0)
                for src_d, dstT in ((f1_d, f1t), (f2_d, f2t)):
                    bulk = big.tile([128, L], f32, tag="strip")
                    nc.sync.dma_start(
                        bulk[:].rearrange("p (s c) -> p s c", c=128),
                        src_d[:, :].rearrange("(s p) c -> p s c", p=128),
                    )
                    for s4 in range(NSTRIP // 4):
                        ps = psA.tile([128, 2048], f32, tag="mm")
                        for q in range(4):
                            s = 4 * s4 + q
                            nc.tensor.transpose(
                                ps[:, 512 * q : 512 * q + 128],
                                bulk[:, 128 * s : 128 * (s + 1)],
                                ident[:],
                            )
                        nc.scalar.copy(
                            dstT[:, 512 * s4 : 512 * (s4 + 1)],
                            ps[:].rearrange("p (q x) -> p q x", x=512)[:, :, 0:128],
                        )

                # a_ij = f1_i . f2_j; the 1/TEMP scale is applied in the PSUM
                # drain (ACT copy with scale=10).
                def orientation_strip(wT, mT, v8a, i8a, s):
                        strip = big.tile([128, L], f32, tag="strip")
                        for h in range(NH):
                            ps = psA.tile([128, 2048], f32, tag="mm")
                            for q in range(4):
                                nc.tensor.matmul(
                                    ps[:, 512 * q : 512 * (q + 1)],
                                    wT[:, 128 * s : 128 * (s + 1)],
                                    mT[:, 2048 * h + 512 * q : 2048 * h + 512 * (q + 1)],
                                    start=True,
                                    stop=True,
                                )
                            nc.scalar.activation(
                                strip[:, 2048 * h : 2048 * (h + 1)],
                                ps[:],
                                AF.Copy,
                                scale=1.0 / TEMP,
                            )
                        nc.vector.max(v8a[:, 8 * s : 8 * s + 8], strip[:])
                        nc.vector.max_index(
                            i8a[:, 8 * s : 8 * s + 8], v8a[:, 8 * s : 8 * s + 8], strip[:]
                        )

                def orientation_pass(wT, mT, v8a, i8a):
                    for s in range(NSTRIP):
                        orientation_strip(wT, mT, v8a, i8a, s)

                def lse8_batched(v8a, out_cols):
                    # out_cols[p, s] = v1 + ln(sum_k exp(v8[s,k] - v1))
                    v3 = v8a[:].rearrange("p (s k) -> p s k", k=8)
                    v1b = v3[:, :, 0:1].broadcast_to([128, NSTRIP, 8])
                    d8 = sm.tile([128, 8 * NSTRIP], f32, tag="d8")
                    d3 = d8[:].rearrange("p (s k) -> p s k", k=8)
                    nc.vector.tensor_tensor(d3, v3, v1b, op=OP.subtract)
                    e8 = sm.tile([128, 8 * NSTRIP], f32, tag="e8")
                    nc.scalar.activation(e8[:], d8[:], AF.Exp)
                    s8 = sm.tile([128, NSTRIP], f32, tag="s8")
                    nc.vector.reduce_sum(
                        s8[:], e8[:].rearrange("p (s k) -> p s k", k=8), axis=X
                    )
                    lg = sm.tile([128, NSTRIP], f32, tag="lg")
                    nc.scalar.activation(lg[:], s8[:], AF.Ln)
                    nc.vector.tensor_tensor(out_cols[:], lg[:], v3[:, :, 0], op=OP.add)

                def build_table(cols, vec_d, tag):
                    # cols [128, 32] (value of row index 128*s+p at [p, s]) ->
                    # replicated table [128, 4096]
                    nc.sync.dma_start(
                        vec_d[:].rearrange("(s p) -> p s", p=128), cols[:]
                    )
                    row = pers.tile([1, L], f32, tag="row")
                    nc.sync.dma_start(row[:], vec_d[:].rearrange("(o n) -> o n", o=1))
                    T = tbl.tile([128, L], f32, tag="tbl")
                    for h in range(NH):
                        ps = psA.tile([128, 2048], f32, tag="mm")
                        for q in range(4):
                            nc.tensor.matmul(
                                ps[:, 512 * q : 512 * (q + 1)],
                                ones1[0:1, :],
                                row[0:1, 2048 * h + 512 * q : 2048 * h + 512 * (q + 1)],
                                start=True,
                                stop=True,
                            )
                        nc.scalar.copy(T[:, 2048 * h : 2048 * (h + 1)], ps[:])
                    return T

                def mask_reduce(g, nidx, tag):
                    # select out[p, n] = g[p, 16*n + p%16], reduce over q
                    selt = big.tile([128, 16 * nidx], f32, tag="strip")
                    g3 = g[:, : 16 * nidx].rearrange("p (n q) -> p n q", q=16)
                    m3 = mask16[:].unsqueeze(1).broadcast_to([128, nidx, 16])
                    s3 = selt[:].rearrange("p (n q) -> p n q", q=16)
                    nc.gpsimd.tensor_tensor(s3, g3, m3, op=OP.mult)
                    outg = sm.tile([128, nidx], f32, tag=tag)
                    nc.vector.reduce_sum(outg[:], s3, axis=X)
                    return outg

                def gather_table(T, idxs, nidx, tag):
                    # per-row gather: out[p, n] = T[p, idxs[p, n]] via the
                    # 16-partition-group indirect_copy + diagonal mask-reduce.
                    g = big.tile([128, 16 * nidx], f32, tag="strip")
                    CH = 64  # ISA limit: <=64 indices per partition per op
                    for c0 in range(0, nidx, CH):
                        c1 = min(c0 + CH, nidx)
                        nc.gpsimd.indirect_copy(
                            g[:, 16 * c0 : 16 * c1], T[:], idxs[:, c0:c1], True
                        )
                    return mask_reduce(g, nidx, tag)

                # ---- pass B: columns (aT strips; weights = f2t tiles)
                orientation_pass(f2t, f1t, cv8a, ci8a)

                # ---- pass A: rows.  Emit a few strips first so ACT stays busy
                # while the B-side lse8/table chain (which waits on all of B's
                # Max8 results) executes; interleave the TC gather chunks.
                def a_strip(s):
                    orientation_strip(f1t, f2t, rv8a, ri8a, s)

                for s in range(4):
                    a_strip(s)
                lse8_batched(cv8a, lsec_c)
                TC = build_table(lsec_c, cvec_d, "tc")
                gTCg = big.tile([128, 16 * 8 * NSTRIP], f32, tag="strip")
                for s in range(4, NSTRIP):
                    a_strip(s)
                    if s in (15, 23, 31):
                        for c in ((0, 1) if s == 15 else (s // 8,)):
                            nc.gpsimd.indirect_copy(
                                gTCg[:, 1024 * c : 1024 * (c + 1)],
                                TC[:],
                                ri8a[:, 64 * c : 64 * (c + 1)],
                                True,
                            )
                lse8_batched(rv8a, lser_c)

                # ---- A-side re-rank: u8 = 2*rv8 - TC[ridx8] - LSE_r
                gTC = mask_reduce(gTCg, 8 * NSTRIP, "gTC")
                t1 = sm.tile([128, 8 * NSTRIP], f32, tag="t1")
                lser_b = lser_c[:].unsqueeze(2).broadcast_to([128, NSTRIP, 8])
                nc.vector.tensor_tensor(
                    t1[:].rearrange("p (s k) -> p s k", k=8),
                    gTC[:].rearrange("p (s k) -> p s k", k=8),
                    lser_b,
                    op=OP.add,
                )
                u8 = sm.tile([128, 8 * NSTRIP], f32, tag="u8")
                nc.vector.scalar_tensor_tensor(
                    u8[:], rv8a[:], 2.0, t1[:], op0=OP.mult, op1=OP.subtract
                )
                nc.vector.reduce_max(
                    ustar_c[:], u8[:].rearrange("p (s k) -> p s k", k=8), axis=X
                )
                eq = sm.tile([128, 8 * NSTRIP], f32, tag="eq")
                ustar_b = ustar_c[:].unsqueeze(2).broadcast_to([128, NSTRIP, 8])
                nc.vector.tensor_tensor(
                    eq[:].rearrange("p (s k) -> p s k", k=8),
                    u8[:].rearrange("p (s k) -> p s k", k=8),
                    ustar_b,
                    op=OP.is_equal,
                )
                jf = sm.tile([128, 8 * NSTRIP], f32, tag="jf")
                nc.vector.tensor_copy(jf[:], ri8a[:])
                jrev = sm.tile([128, 8 * NSTRIP], f32, tag="jrev")
                nc.vector.tensor_scalar(
                    jrev[:], jf[:], -1.0, float(L), op0=OP.mult, op1=OP.add
                )
                sel2 = sm.tile([128, 8 * NSTRIP], f32, tag="sel2")
                nc.vector.tensor_tensor(sel2[:], eq[:], jrev[:], op=OP.mult)
                jenc = sm.tile([128, NSTRIP], f32, tag="jenc")
                nc.vector.reduce_max(
                    jenc[:], sel2[:].rearrange("p (s k) -> p s k", k=8), axis=X
                )
                nc.vector.tensor_scalar(
                    jstar_c[:], jenc[:], -1.0, float(L), op0=OP.mult, op1=OP.add
                )
                nc.vector.tensor_copy(jst_u16[:], jstar_c[:])

                # ---- B-side re-rank: colmax of u at each column
                TR = build_table(lser_c, rvec_d, "tr2")
                gTR = gather_table(TR, ci8a, 8 * NSTRIP, "gTR")
                t2 = sm.tile([128, 8 * NSTRIP], f32, tag="t2")
                lsec_b = lsec_c[:].unsqueeze(2).broadcast_to([128, NSTRIP, 8])
                nc.vector.tensor_tensor(
                    t2[:].rearrange("p (s k) -> p s k", k=8),
                    gTR[:].rearrange("p (s k) -> p s k", k=8),
                    lsec_b,
                    op=OP.add,
                )
                uB8 = sm.tile([128, 8 * NSTRIP], f32, tag="uB8")
                nc.vector.scalar_tensor_tensor(
                    uB8[:], cv8a[:], 2.0, t2[:], op0=OP.mult, op1=OP.subtract
                )
                nc.vector.reduce_max(
                    cmw_c[:], uB8[:].rearrange("p (s k) -> p s k", k=8), axis=X
                )

                # ---- mutual + threshold
                CMW = build_table(cmw_c, wvec_d, "cmwt")
                cmj = gather_table(CMW, jst_u16, NSTRIP, "cmj")
                dd = sm.tile([128, NSTRIP], f32, tag="dd")
                nc.vector.tensor_tensor(dd[:], ustar_c[:], cmj[:], op=OP.subtract)
                m1 = sm.tile([128, NSTRIP], f32, tag="m1")
                nc.vector.tensor_scalar(m1[:], dd[:], EPS_MUT, None, op0=OP.is_le)
                m2 = sm.tile([128, NSTRIP], f32, tag="m2")
                nc.vector.tensor_scalar(m2[:], dd[:], -EPS_MUT, None, op0=OP.is_ge)
                mut = sm.tile([128, NSTRIP], f32, tag="mut")
                nc.vector.tensor_tensor(mut[:], m1[:], m2[:], op=OP.mult)
                nc.vector.scalar_tensor_tensor(
                    keep_c[:], ustar_c[:], LN_THRESH, mut[:],
                    op0=OP.is_gt, op1=OP.mult,
                )

                # ---- jsel = keep ? j* : L  (column L of f2t is zero)
                jself = sm.tile([128, NSTRIP], f32, tag="jself")
                nc.vector.scalar_tensor_tensor(
                    jself[:], jstar_c[:], -float(L), keep_c[:],
                    op0=OP.add, op1=OP.mult,
                )
                jsel_u16 = sm.tile([128, NSTRIP], u16, tag="jselu")
                nc.vector.tensor_scalar(
                    jsel_u16[:], jself[:], float(L), None, op0=OP.add
                )

                # ---- gather f2.T columns at jsel via indirect_copy
                # wrapped shared index list: idxsw[p, slot] = jsel_(16*slot + p%16),
                # identical across the 8 16-partition groups.
                nc.sync.dma_start(
                    jvec_d[:].rearrange("(s p) -> p s", p=128), jsel_u16[:]
                )
                for g in range(8):
                    nc.sync.dma_start(
                        idxsw[16 * g : 16 * (g + 1), :],
                        jvec_d[:].rearrange("(s p) -> p s", p=16),
                    )
                f2gT = big.tile([128, L], f32, tag="strip")
                outT = big.tile([128, L], f32, tag="strip")
                for c0 in range(0, 256, 64):
                    lo, hi = 16 * c0, 16 * (c0 + 64)
                    nc.gpsimd.indirect_copy(
                        f2gT[:, lo:hi], f2t[:], idxsw[:, c0 : c0 + 64], True
                    )
                    # out[c, i] = f1t[c, i] - f2gT[c, i], store per chunk
                    nc.vector.tensor_tensor(
                        outT[:, lo:hi], f1t[:, lo:hi], f2gT[:, lo:hi],
                        op=OP.subtract,
                    )
                    nc.sync.dma_start(out_d[:, lo:hi], outT[:, lo:hi])

    if hasattr(nc, "finalize"):
        nc.finalize()
    return nc


def _get_nc():
    if "nc" not in _NC_CACHE:
        _NC_CACHE["nc"] = _build_nc()
    return _NC_CACHE["nc"]


def _host_inputs(f1b, f2b):
    ident = np.eye(128, dtype=np.float32)
    mask16 = (
        np.arange(16)[None, :] == (np.arange(128) % 16)[:, None]
    ).astype(np.float32)
    return {"f1": f1b, "f2": f2b, "ident": ident, "mask16": mask16}


def run(feature1, feature2, trace=False):
    from concourse.bass_utils import run_bass_kernel_spmd

    f1 = np.ascontiguousarray(np.asarray(feature1), dtype=np.float32)
    f2 = np.ascontiguousarray(np.asarray(feature2), dtype=np.float32)
    assert f1.shape == (B, L, C) and f2.shape == (B, L, C)
    nc = _get_nc()
    in_maps = [_host_inputs(f1[b], f2[b]) for b in range(B)]
    res = run_bass_kernel_spmd(nc, in_maps, core_ids=list(range(B)), trace=trace)
    out = np.stack([res.results[b]["out"].reshape(C, 64, 64) for b in range(B)])
    return out.astype(np.float32), res


def kernel(feature1, feature2, h=64, w=64):
    out, _ = run(feature1, feature2, trace=False)
    return out

